# revision 23
# baseline (speedup 1.0000x reference)
"""ContextBasedLinear Trainium2 kernel.

Computes out = mu * x + gamma * sum(x, axis=1, keepdims=True) for
x: [64, 1024, 512] f32, mu/gamma: [1] f32.

Sharding: data-parallel on the batch dim — 8 batches per core on 8
NeuronCores; mu/gamma replicated. No cross-core comms needed.

Per-core program (x_c: [8, 1024, 512]):
  Each batch's [1024, 512] lives in SBUF as [128, 4096]: partition p
  holds set rows 8p..8p+7 (16 KB contiguous per partition).

  colsum (reduce over 1024 set rows), already scaled by gamma and
  replicated to all partitions, via a 2-level DVE add tree plus two
  accumulating PE matmuls with an all-gamma stationary matrix:
    t = xt[:, 0:2048] + xt[:, 2048:4096]        (DVE, [128,2048])
    u = t[:, 0:1024]  + t[:, 1024:2048]         (DVE, [128,1024])
    psum_b[128,512] = gmat.T @ u[:, 0:512]      (PE, start)
                    + gmat.T @ u[:, 512:1024]   (PE, stop)
  with gmat[128,128] = gamma * ones: out[m,d] = gamma * sum_k u[k,d]
  for every output partition m — the broadcast IS the matmul, so
  there is no psum->sbuf copy and no second rank-1 matmul on the
  critical path.  The naive alternative (8 fp32 colsum matmuls per
  batch) costs ~2.1 us each and saturates the PE for the whole
  kernel; offloading the tree to GpSimd loses too: concurrent GpSimd
  tensor ops contend for SBUF and slow DVE ops 4-7x (measured).
  Engines: DVE ~8.2 us/batch (tree + STT), PE ~4.2 (2 matmuls),
  ACT/SP pure DMA issuers, GpSimd idle.

  out = (x * mu) + psum_b in one fused DVE scalar_tensor_tensor pass
  per chunk (psum_b read through a step-0 broadcast AP).

  Software pipeline: iteration i loads batch i, runs PE matmuls and
  STT of batch i-1 (per-engine FIFO order matches dep maturity), and
  runs the tree of batch i.

  DMA schedule: both HWDGE rings (SP/sync and ACT/scalar) sustain
  ~430 GB/s combined.  Batch i loads as two 1 MiB halves, h0 on sync,
  h1 on scalar, so batch i is fully resident ~5 us after its DMAs
  start; stores mirror that (h0 chunks on sync, h1 on scalar), which
  matches the 1 MiB / ~4.7 us cadence the DVE produces output at.
  (Coarser 2 MiB full-batch stores were tried and lose ~9 us: the
  first store's issue head-of-line-blocks the next load issue on its
  ring while both STT halves finish.)  From iteration 3 on each
  iteration flushes one store per ring (batch i-3), so both rings
  stay busy end to end; the last three batches store as 0.5 MiB
  quarters so the drain tail ends in quarter-sized steps.  mu/gamma
  ride the GpSimd software-DGE queue: each 4 B HBM DMA pays a
  ~2.4 us completion round-trip that would stall either HWDGE ring
  ahead of its first 1 MiB x load.
"""

import collections

import numpy as np

import concourse.bacc as bacc
import concourse.mybir as mybir
import concourse.tile as tile
from concourse.bass_utils import run_bass_kernel_spmd

N_CORES = 8
B_FULL = 64
B_PER = B_FULL // N_CORES  # 8 batches per core
N_SET = 1024
D = 512
P = 128
R = N_SET // P  # 8 set-rows per partition
F = R * D  # 4096 free elems per partition
HF = F // 2  # 2048 per half
QF = F // 4  # 1024 per quarter

STORE_LAG = 3  # iteration at which per-ring store flushing starts
# Batches stored as 0.5 MiB quarters: 0-1 so the first ring store slots
# (~28 us) have chunks ready ~4 us early (the first-store transition is
# where HBM-contended cores collapse), 5-7 so the drain tail ends in
# quarter-sized steps.
QUARTER_BATCHES = (0, 1, B_PER - 3, B_PER - 2, B_PER - 1)

_cache = {}


def build_nc():
    if "nc" in _cache:
        return _cache["nc"]
    f32 = mybir.dt.float32
    nc = bacc.Bacc(
        "TRN2", target_bir_lowering=False, debug=False, num_devices=N_CORES
    )
    x_d = nc.dram_tensor("x", [B_PER, N_SET, D], f32, kind="ExternalInput").ap()
    mu_d = nc.dram_tensor("mu", [1], f32, kind="ExternalInput").ap()
    gamma_d = nc.dram_tensor("gamma", [1], f32, kind="ExternalInput").ap()
    out_d = nc.dram_tensor("out", [B_PER, N_SET, D], f32, kind="ExternalOutput").ap()

    with tile.TileContext(nc) as tc:
        with (
            tc.tile_pool(name="consts", bufs=1) as consts,
            tc.tile_pool(name="xp", bufs=5) as xp,
            tc.tile_pool(name="tpool", bufs=2) as tpool,
            tc.tile_pool(name="upool", bufs=2) as upool,
            tc.tile_pool(name="op", bufs=6) as op,
            tc.tile_pool(name="oq", bufs=6) as oq,
            tc.tile_pool(name="psc", bufs=1, space="PSUM") as psc,
            tc.tile_pool(name="pb", bufs=2, space="PSUM") as pb,
        ):
            # ---- constants ----
            ones_row = consts.tile([1, P], f32)
            nc.vector.memset(ones_row, 1.0)
            ones_mat = consts.tile([P, P], f32)
            nc.vector.memset(ones_mat, 1.0)
            # mu/gamma ride the GpSimd SWDGE queue (engine otherwise idle)
            mg_sb = consts.tile([1, 2], f32)
            nc.gpsimd.dma_start(mg_sb[:, 0:1], mu_d[None, :])
            nc.gpsimd.dma_start(mg_sb[:, 1:2], gamma_d[None, :])
            # replicate mu/gamma to all 128 partitions via rank-1 matmul
            psum_mg = psc.tile([P, 2], f32, tag="psmg")
            nc.tensor.matmul(
                psum_mg, lhsT=ones_row[:], rhs=mg_sb[:], start=True, stop=True
            )
            mg_col = consts.tile([P, 2], f32)
            nc.vector.tensor_copy(mg_col, psum_mg)
            mu_col = mg_col[:, 0:1]
            # gmat[128,128] = gamma * ones: colsum matmuls with gmat
            # stationary yield gamma*colsum broadcast to every partition.
            gmat = consts.tile([P, P], f32)
            nc.vector.tensor_scalar_mul(gmat, ones_mat, mg_col[:, 1:2])

            # pending stores per ring: deque of (dram_view, sbuf_tile)
            pend = {"sync": collections.deque(), "scalar": collections.deque()}

            def flush(ring, n):
                eng = nc.sync if ring == "sync" else nc.scalar
                for _ in range(n):
                    if not pend[ring]:
                        return
                    view, t = pend[ring].popleft()
                    eng.dma_start(view, t)

            state = {}  # batch -> (xt, u tile)

            for i in range(B_PER + 1):
                # -- load batch i: two 1 MiB halves, one per ring.
                # Batch 0 loads in quarters with the tree's first operand
                # pair (q0, q2) up front, so t_lo starts at the half-way
                # point of the load instead of the end. --
                if i < B_PER:
                    x_view = x_d[i].rearrange("(p r) d -> p (r d)", p=P)
                    xt = xp.tile([P, F], f32, tag="xt")
                    if i == 0:
                        nc.sync.dma_start(xt[:, 0:QF], x_view[:, 0:QF])
                        nc.scalar.dma_start(
                            xt[:, HF : HF + QF], x_view[:, HF : HF + QF]
                        )
                        nc.sync.dma_start(xt[:, QF:HF], x_view[:, QF:HF])
                        nc.scalar.dma_start(
                            xt[:, HF + QF : F], x_view[:, HF + QF : F]
                        )
                    else:
                        nc.sync.dma_start(xt[:, 0:HF], x_view[:, 0:HF])
                        nc.scalar.dma_start(xt[:, HF:F], x_view[:, HF:F])

                # -- steady-state store flush; the first two flush
                # iterations pop two quarter-chunks per ring so the
                # per-iteration flushed bytes match the 1 MiB cadence --
                if i >= STORE_LAG:
                    n = 2 if i in (STORE_LAG, STORE_LAG + 1) else 1
                    flush("sync", n)
                    flush("scalar", n)

                # -- back half for batch i-1: matmuls + STT + stores --
                if i >= 1:
                    b = i - 1
                    bxt, u = state.pop(b)
                    psum_b = pb.tile([P, D], f32, tag="psb")
                    nc.tensor.matmul(
                        psum_b, lhsT=gmat[:], rhs=u[:, 0:D],
                        start=True, stop=False,
                    )
                    nc.tensor.matmul(
                        psum_b, lhsT=gmat[:], rhs=u[:, D:QF],
                        start=False, stop=True,
                    )
                    o_view = out_d[b].rearrange("(p r) d -> p (r d)", p=P)
                    nq = 2 if b in QUARTER_BATCHES else 1
                    fq = HF // nq
                    rq = (R // 2) // nq
                    for h in range(2):
                        for q in range(nq):
                            lo = h * HF + q * fq
                            pool = oq if nq == 2 else op
                            ot = pool.tile([P, fq], f32, tag="ot")
                            nc.vector.scalar_tensor_tensor(
                                out=ot[:].rearrange("p (r d) -> p r d", r=rq),
                                in0=bxt[:, lo : lo + fq].rearrange(
                                    "p (r d) -> p r d", r=rq
                                ),
                                scalar=mu_col,
                                in1=psum_b[:, None, :].broadcast_to([P, rq, D]),
                                op0=mybir.AluOpType.mult,
                                op1=mybir.AluOpType.add,
                            )
                            ring = "sync" if h == 0 else "scalar"
                            pend[ring].append((o_view[:, lo : lo + fq], ot))

                # -- front half for batch i: 2-level DVE add tree --
                if i < B_PER:
                    t = tpool.tile([P, HF], f32, tag="t")
                    if i == 0:
                        # split t so t_lo only needs quarters q0+q2
                        nc.vector.tensor_add(
                            t[:, 0:QF], xt[:, 0:QF], xt[:, HF : HF + QF]
                        )
                        nc.vector.tensor_add(
                            t[:, QF:HF], xt[:, QF:HF], xt[:, HF + QF : F]
                        )
                    else:
                        nc.vector.tensor_add(t, xt[:, 0:HF], xt[:, HF:F])
                    u = upool.tile([P, QF], f32, tag="u")
                    nc.vector.tensor_add(u, t[:, 0:QF], t[:, QF:HF])
                    state[i] = (xt, u)

            # drain remaining stores, alternating so both rings share
            while pend["sync"] or pend["scalar"]:
                flush("sync", 1)
                flush("scalar", 1)

    nc.compile()
    _cache["nc"] = nc
    return nc


def run_spmd(x, mu, gamma, **spmd_kwargs):
    nc = build_nc()
    x = np.ascontiguousarray(x, dtype=np.float32)
    mu = np.ascontiguousarray(mu, dtype=np.float32)
    gamma = np.ascontiguousarray(gamma, dtype=np.float32)
    in_maps = [
        {"x": x[c * B_PER : (c + 1) * B_PER], "mu": mu, "gamma": gamma}
        for c in range(N_CORES)
    ]
    return run_bass_kernel_spmd(nc, in_maps, list(range(N_CORES)), **spmd_kwargs)


def kernel(x, mu, gamma):
    res = run_spmd(x, mu, gamma)
    out = np.concatenate([r["out"] for r in res.results], axis=0)
    return out


# revision 30
# speedup vs baseline: 1.0106x; 1.0106x over previous
"""ContextBasedLinear Trainium2 kernel.

Computes out = mu * x + gamma * sum(x, axis=1, keepdims=True) for
x: [64, 1024, 512] f32, mu/gamma: [1] f32.

Sharding: data-parallel on the batch dim — 8 batches per core on 8
NeuronCores; mu/gamma replicated. No cross-core comms needed.

Per-core program (x_c: [8, 1024, 512]):
  Each batch's [1024, 512] lives in SBUF as [128, 4096]: partition p
  holds set rows 8p..8p+7 (16 KB contiguous per partition).

  colsum (reduce over 1024 set rows), already scaled by gamma and
  replicated to all partitions, via a 2-level DVE add tree plus two
  accumulating PE matmuls with an all-gamma stationary matrix:
    t = xt[:, 0:2048] + xt[:, 2048:4096]        (DVE, [128,2048])
    u = t[:, 0:1024]  + t[:, 1024:2048]         (DVE, [128,1024])
    psum_b[128,512] = gmat.T @ u[:, 0:512]      (PE, start)
                    + gmat.T @ u[:, 512:1024]   (PE, stop)
  with gmat[128,128] = gamma * ones: out[m,d] = gamma * sum_k u[k,d]
  for every output partition m — the broadcast IS the matmul, so
  there is no psum->sbuf copy and no second rank-1 matmul on the
  critical path.  The naive alternative (8 fp32 colsum matmuls per
  batch) costs ~2.1 us each and saturates the PE for the whole
  kernel; offloading the tree to GpSimd loses too: concurrent GpSimd
  tensor ops contend for SBUF and slow DVE ops 4-7x (measured).
  Engines: DVE ~8.2 us/batch (tree + STT), PE ~4.2 (2 matmuls),
  ACT/SP pure DMA issuers, GpSimd idle.

  out = (x * mu) + psum_b in one fused DVE scalar_tensor_tensor pass
  per chunk (psum_b read through a step-0 broadcast AP).

  Software pipeline: iteration i loads batch i, runs PE matmuls and
  STT of batch i-1 (per-engine FIFO order matches dep maturity), and
  runs the tree of batch i.

  DMA schedule: both HWDGE rings (SP/sync and ACT/scalar) sustain
  ~430 GB/s combined.  Batch i loads as two 1 MiB halves, h0 on sync,
  h1 on scalar, so batch i is fully resident ~5 us after its DMAs
  start; stores mirror that (h0 chunks on sync, h1 on scalar), which
  matches the 1 MiB / ~4.7 us cadence the DVE produces output at.
  (Coarser 2 MiB full-batch stores were tried and lose ~9 us: the
  first store's issue head-of-line-blocks the next load issue on its
  ring while both STT halves finish.)  From iteration 3 on each
  iteration flushes one store per ring (batch i-3), so both rings
  stay busy end to end; the last three batches store as 0.5 MiB
  quarters so the drain tail ends in quarter-sized steps.  mu/gamma
  ride the GpSimd software-DGE queue: each 4 B HBM DMA pays a
  ~2.4 us completion round-trip that would stall either HWDGE ring
  ahead of its first 1 MiB x load.
"""

import collections

import numpy as np

import concourse.bacc as bacc
import concourse.mybir as mybir
import concourse.tile as tile
from concourse.bass_utils import run_bass_kernel_spmd

N_CORES = 8
B_FULL = 64
B_PER = B_FULL // N_CORES  # 8 batches per core
N_SET = 1024
D = 512
P = 128
R = N_SET // P  # 8 set-rows per partition
F = R * D  # 4096 free elems per partition
HF = F // 2  # 2048 per half
QF = F // 4  # 1024 per quarter

STORE_LAG = 3  # iteration at which per-ring store flushing starts
QUARTER_BATCHES = (B_PER - 3, B_PER - 2, B_PER - 1)  # stored as quarters
# Middle batches whose colsum matmuls consume t (4 chunks) directly,
# skipping the u-level DVE add: output production is DVE-serial-bound,
# so trimming ~1.2 us/batch off the DVE ends the store drain earlier.
# Head (b0) and tail (b5-b7) batches keep the 2-matmul u path: their
# psum_b sits on the first-store / last-store critical path and the
# 4-chunk matmul pair costs ~4 us more PE latency.
PE_DIRECT = (1, 2, 3, 4)

_cache = {}


def build_nc():
    if "nc" in _cache:
        return _cache["nc"]
    f32 = mybir.dt.float32
    nc = bacc.Bacc(
        "TRN2", target_bir_lowering=False, debug=False, num_devices=N_CORES
    )
    x_d = nc.dram_tensor("x", [B_PER, N_SET, D], f32, kind="ExternalInput").ap()
    mu_d = nc.dram_tensor("mu", [1], f32, kind="ExternalInput").ap()
    gamma_d = nc.dram_tensor("gamma", [1], f32, kind="ExternalInput").ap()
    out_d = nc.dram_tensor("out", [B_PER, N_SET, D], f32, kind="ExternalOutput").ap()

    with tile.TileContext(nc) as tc:
        with (
            tc.tile_pool(name="consts", bufs=1) as consts,
            tc.tile_pool(name="xp", bufs=5) as xp,
            tc.tile_pool(name="tpool", bufs=3) as tpool,
            tc.tile_pool(name="upool", bufs=2) as upool,
            tc.tile_pool(name="op", bufs=6) as op,
            tc.tile_pool(name="oq", bufs=6) as oq,
            tc.tile_pool(name="psc", bufs=1, space="PSUM") as psc,
            tc.tile_pool(name="pb", bufs=2, space="PSUM") as pb,
        ):
            # ---- constants ----
            ones_row = consts.tile([1, P], f32)
            nc.vector.memset(ones_row, 1.0)
            ones_mat = consts.tile([P, P], f32)
            nc.vector.memset(ones_mat, 1.0)
            # mu/gamma ride the GpSimd SWDGE queue (engine otherwise idle)
            mg_sb = consts.tile([1, 2], f32)
            nc.gpsimd.dma_start(mg_sb[:, 0:1], mu_d[None, :])
            nc.gpsimd.dma_start(mg_sb[:, 1:2], gamma_d[None, :])
            # replicate mu/gamma to all 128 partitions via rank-1 matmul
            psum_mg = psc.tile([P, 2], f32, tag="psmg")
            nc.tensor.matmul(
                psum_mg, lhsT=ones_row[:], rhs=mg_sb[:], start=True, stop=True
            )
            mg_col = consts.tile([P, 2], f32)
            nc.vector.tensor_copy(mg_col, psum_mg)
            mu_col = mg_col[:, 0:1]
            # gmat[128,128] = gamma * ones: colsum matmuls with gmat
            # stationary yield gamma*colsum broadcast to every partition.
            gmat = consts.tile([P, P], f32)
            nc.vector.tensor_scalar_mul(gmat, ones_mat, mg_col[:, 1:2])

            # pending stores per ring: deque of (dram_view, sbuf_tile)
            pend = {"sync": collections.deque(), "scalar": collections.deque()}

            def flush(ring, n):
                eng = nc.sync if ring == "sync" else nc.scalar
                for _ in range(n):
                    if not pend[ring]:
                        return
                    view, t = pend[ring].popleft()
                    eng.dma_start(view, t)

            state = {}  # batch -> (xt, u tile)

            for i in range(B_PER + 1):
                # -- load batch i: two 1 MiB halves, one per ring --
                if i < B_PER:
                    x_view = x_d[i].rearrange("(p r) d -> p (r d)", p=P)
                    xt = xp.tile([P, F], f32, tag="xt")
                    nc.sync.dma_start(xt[:, 0:HF], x_view[:, 0:HF])
                    nc.scalar.dma_start(xt[:, HF:F], x_view[:, HF:F])

                # -- steady-state store flush (one per ring) --
                if i >= STORE_LAG:
                    flush("sync", 1)
                    flush("scalar", 1)

                # -- back half for batch i-1: matmuls + STT + stores --
                if i >= 1:
                    b = i - 1
                    bxt, red, nmm = state.pop(b)
                    psum_b = pb.tile([P, D], f32, tag="psb")
                    for j in range(nmm):
                        nc.tensor.matmul(
                            psum_b, lhsT=gmat[:],
                            rhs=red[:, j * D : (j + 1) * D],
                            start=(j == 0), stop=(j == nmm - 1),
                        )
                    o_view = out_d[b].rearrange("(p r) d -> p (r d)", p=P)
                    nq = 2 if b in QUARTER_BATCHES else 1
                    fq = HF // nq
                    rq = (R // 2) // nq
                    for h in range(2):
                        for q in range(nq):
                            lo = h * HF + q * fq
                            pool = oq if nq == 2 else op
                            ot = pool.tile([P, fq], f32, tag="ot")
                            nc.vector.scalar_tensor_tensor(
                                out=ot[:].rearrange("p (r d) -> p r d", r=rq),
                                in0=bxt[:, lo : lo + fq].rearrange(
                                    "p (r d) -> p r d", r=rq
                                ),
                                scalar=mu_col,
                                in1=psum_b[:, None, :].broadcast_to([P, rq, D]),
                                op0=mybir.AluOpType.mult,
                                op1=mybir.AluOpType.add,
                            )
                            ring = "sync" if h == 0 else "scalar"
                            pend[ring].append((o_view[:, lo : lo + fq], ot))

                # -- front half for batch i: DVE add tree (1 or 2 levels) --
                if i < B_PER:
                    t = tpool.tile([P, HF], f32, tag="t")
                    nc.vector.tensor_add(t, xt[:, 0:HF], xt[:, HF:F])
                    if i in PE_DIRECT:
                        state[i] = (xt, t, 4)
                    else:
                        u = upool.tile([P, QF], f32, tag="u")
                        nc.vector.tensor_add(u, t[:, 0:QF], t[:, QF:HF])
                        state[i] = (xt, u, 2)

            # drain remaining stores, alternating so both rings share
            while pend["sync"] or pend["scalar"]:
                flush("sync", 1)
                flush("scalar", 1)

    nc.compile()
    _cache["nc"] = nc
    return nc


def run_spmd(x, mu, gamma, **spmd_kwargs):
    nc = build_nc()
    x = np.ascontiguousarray(x, dtype=np.float32)
    mu = np.ascontiguousarray(mu, dtype=np.float32)
    gamma = np.ascontiguousarray(gamma, dtype=np.float32)
    in_maps = [
        {"x": x[c * B_PER : (c + 1) * B_PER], "mu": mu, "gamma": gamma}
        for c in range(N_CORES)
    ]
    return run_bass_kernel_spmd(nc, in_maps, list(range(N_CORES)), **spmd_kwargs)


def kernel(x, mu, gamma):
    res = run_spmd(x, mu, gamma)
    out = np.concatenate([r["out"] for r in res.results], axis=0)
    return out


# revision 34
# speedup vs baseline: 1.0343x; 1.0235x over previous
"""ContextBasedLinear Trainium2 kernel.

Computes out = mu * x + gamma * sum(x, axis=1, keepdims=True) for
x: [64, 1024, 512] f32, mu/gamma: [1] f32.

Sharding: data-parallel on the batch dim — 8 batches per core on 8
NeuronCores; mu/gamma replicated. No cross-core comms needed.

Per-core program (x_c: [8, 1024, 512]):
  Each batch's [1024, 512] lives in SBUF as [128, 4096]: partition p
  holds set rows 8p..8p+7 (16 KB contiguous per partition).

  colsum (reduce over 1024 set rows), already scaled by gamma and
  replicated to all partitions, via a 2-level DVE add tree plus two
  accumulating PE matmuls with an all-gamma stationary matrix:
    t = xt[:, 0:2048] + xt[:, 2048:4096]        (DVE, [128,2048])
    u = t[:, 0:1024]  + t[:, 1024:2048]         (DVE, [128,1024])
    psum_b[128,512] = gmat.T @ u[:, 0:512]      (PE, start)
                    + gmat.T @ u[:, 512:1024]   (PE, stop)
  with gmat[128,128] = gamma * ones: out[m,d] = gamma * sum_k u[k,d]
  for every output partition m — the broadcast IS the matmul, so
  there is no psum->sbuf copy and no second rank-1 matmul on the
  critical path.  The naive alternative (8 fp32 colsum matmuls per
  batch) costs ~2.1 us each and saturates the PE for the whole
  kernel; offloading the tree to GpSimd loses too: concurrent GpSimd
  tensor ops contend for SBUF and slow DVE ops 4-7x (measured).
  Engines: DVE ~8.2 us/batch (tree + STT), PE ~4.2 (2 matmuls),
  ACT/SP pure DMA issuers, GpSimd idle.

  out = (x * mu) + psum_b in one fused DVE scalar_tensor_tensor pass
  per chunk (psum_b read through a step-0 broadcast AP).

  Software pipeline: iteration i loads batch i, runs PE matmuls and
  STT of batch i-1 (per-engine FIFO order matches dep maturity), and
  runs the tree of batch i.

  DMA schedule: both HWDGE rings (SP/sync and ACT/scalar) sustain
  ~430 GB/s combined.  Batch i loads as two 1 MiB halves, h0 on sync,
  h1 on scalar, so batch i is fully resident ~5 us after its DMAs
  start; stores mirror that (h0 chunks on sync, h1 on scalar), which
  matches the 1 MiB / ~4.7 us cadence the DVE produces output at.
  (Coarser 2 MiB full-batch stores were tried and lose ~9 us: the
  first store's issue head-of-line-blocks the next load issue on its
  ring while both STT halves finish.)  From iteration 3 on each
  iteration flushes one store per ring (batch i-3), so both rings
  stay busy end to end; the last three batches store as 0.5 MiB
  quarters so the drain tail ends in quarter-sized steps.  mu/gamma
  ride the GpSimd software-DGE queue: each 4 B HBM DMA pays a
  ~2.4 us completion round-trip that would stall either HWDGE ring
  ahead of its first 1 MiB x load.
"""

import collections

import numpy as np

import concourse.bacc as bacc
import concourse.mybir as mybir
import concourse.tile as tile
from concourse.bass_utils import run_bass_kernel_spmd

N_CORES = 8
B_FULL = 64
B_PER = B_FULL // N_CORES  # 8 batches per core
N_SET = 1024
D = 512
P = 128
R = N_SET // P  # 8 set-rows per partition
F = R * D  # 4096 free elems per partition
HF = F // 2  # 2048 per half
QF = F // 4  # 1024 per quarter

STORE_LAG = 3  # iteration at which per-ring store flushing starts
QUARTER_BATCHES = (B_PER - 3, B_PER - 2, B_PER - 1)  # stored as quarters

_cache = {}


def build_nc():
    if "nc" in _cache:
        return _cache["nc"]
    f32 = mybir.dt.float32
    nc = bacc.Bacc(
        "TRN2", target_bir_lowering=False, debug=False, num_devices=N_CORES
    )
    x_d = nc.dram_tensor("x", [B_PER, N_SET, D], f32, kind="ExternalInput").ap()
    mu_d = nc.dram_tensor("mu", [1], f32, kind="ExternalInput").ap()
    gamma_d = nc.dram_tensor("gamma", [1], f32, kind="ExternalInput").ap()
    out_d = nc.dram_tensor("out", [B_PER, N_SET, D], f32, kind="ExternalOutput").ap()

    with tile.TileContext(nc) as tc:
        with (
            tc.tile_pool(name="consts", bufs=1) as consts,
            tc.tile_pool(name="xp", bufs=5) as xp,
            tc.tile_pool(name="tpool", bufs=2) as tpool,
            tc.tile_pool(name="upool", bufs=2) as upool,
            tc.tile_pool(name="op", bufs=6) as op,
            tc.tile_pool(name="oq", bufs=6) as oq,
            tc.tile_pool(name="psc", bufs=1, space="PSUM") as psc,
            tc.tile_pool(name="psd", bufs=1, space="PSUM") as psd,
            tc.tile_pool(name="pb", bufs=2, space="PSUM") as pb,
        ):
            # ---- constants ----
            ones_row = consts.tile([1, P], f32)
            nc.vector.memset(ones_row, 1.0)
            ones_mat = consts.tile([P, P], f32)
            nc.vector.memset(ones_mat, 1.0)
            # ---- PE p-state warm-up ----
            # fp32 matmul passes run ~1055-1222 ns cold vs ~592 ns warm,
            # and batch 0's colsum pair sits on the first-store critical
            # chain.  The PE is idle for the first ~20 us, so burn a few
            # dependency-free matmuls right after the preamble to ramp
            # the clock before the real pairs issue.  The [1,1] copy
            # below keeps the result live so nothing eliminates them.
            ones_d = consts.tile([1, D], f32)
            nc.vector.memset(ones_d, 1.0)
            psum_warm = psd.tile([P, D], f32, tag="warm")
            for _ in range(3):
                nc.tensor.matmul(
                    psum_warm, lhsT=ones_row[:], rhs=ones_d[:],
                    start=True, stop=True,
                )
            warm_sink = consts.tile([1, 1], f32)
            nc.vector.tensor_copy(warm_sink, psum_warm[0:1, 0:1])
            # mu/gamma ride the GpSimd SWDGE queue (engine otherwise idle)
            mg_sb = consts.tile([1, 2], f32)
            nc.gpsimd.dma_start(mg_sb[:, 0:1], mu_d[None, :])
            nc.gpsimd.dma_start(mg_sb[:, 1:2], gamma_d[None, :])
            # replicate mu/gamma to all 128 partitions via rank-1 matmul
            psum_mg = psc.tile([P, 2], f32, tag="psmg")
            nc.tensor.matmul(
                psum_mg, lhsT=ones_row[:], rhs=mg_sb[:], start=True, stop=True
            )
            mg_col = consts.tile([P, 2], f32)
            nc.vector.tensor_copy(mg_col, psum_mg)
            mu_col = mg_col[:, 0:1]
            # gmat[128,128] = gamma * ones: colsum matmuls with gmat
            # stationary yield gamma*colsum broadcast to every partition.
            gmat = consts.tile([P, P], f32)
            nc.vector.tensor_scalar_mul(gmat, ones_mat, mg_col[:, 1:2])

            # pending stores per ring: deque of (dram_view, sbuf_tile)
            pend = {"sync": collections.deque(), "scalar": collections.deque()}

            def flush(ring, n):
                eng = nc.sync if ring == "sync" else nc.scalar
                for _ in range(n):
                    if not pend[ring]:
                        return
                    view, t = pend[ring].popleft()
                    eng.dma_start(view, t)

            state = {}  # batch -> (xt, u tile)

            for i in range(B_PER + 1):
                # -- load batch i: two 1 MiB halves, one per ring --
                if i < B_PER:
                    x_view = x_d[i].rearrange("(p r) d -> p (r d)", p=P)
                    xt = xp.tile([P, F], f32, tag="xt")
                    nc.sync.dma_start(xt[:, 0:HF], x_view[:, 0:HF])
                    nc.scalar.dma_start(xt[:, HF:F], x_view[:, HF:F])

                # -- steady-state store flush (one per ring) --
                if i >= STORE_LAG:
                    flush("sync", 1)
                    flush("scalar", 1)

                # -- back half for batch i-1: matmuls + STT + stores --
                if i >= 1:
                    b = i - 1
                    bxt, red, nmm = state.pop(b)
                    psum_b = pb.tile([P, D], f32, tag="psb")
                    for j in range(nmm):
                        nc.tensor.matmul(
                            psum_b, lhsT=gmat[:],
                            rhs=red[:, j * D : (j + 1) * D],
                            start=(j == 0), stop=(j == nmm - 1),
                        )
                    o_view = out_d[b].rearrange("(p r) d -> p (r d)", p=P)
                    nq = 2 if b in QUARTER_BATCHES else 1
                    fq = HF // nq
                    rq = (R // 2) // nq
                    for h in range(2):
                        for q in range(nq):
                            lo = h * HF + q * fq
                            pool = oq if nq == 2 else op
                            ot = pool.tile([P, fq], f32, tag="ot")
                            nc.vector.scalar_tensor_tensor(
                                out=ot[:].rearrange("p (r d) -> p r d", r=rq),
                                in0=bxt[:, lo : lo + fq].rearrange(
                                    "p (r d) -> p r d", r=rq
                                ),
                                scalar=mu_col,
                                in1=psum_b[:, None, :].broadcast_to([P, rq, D]),
                                op0=mybir.AluOpType.mult,
                                op1=mybir.AluOpType.add,
                            )
                            ring = "sync" if h == 0 else "scalar"
                            pend[ring].append((o_view[:, lo : lo + fq], ot))

                # -- front half for batch i: 2-level DVE add tree --
                if i < B_PER:
                    t = tpool.tile([P, HF], f32, tag="t")
                    nc.vector.tensor_add(t, xt[:, 0:HF], xt[:, HF:F])
                    u = upool.tile([P, QF], f32, tag="u")
                    nc.vector.tensor_add(u, t[:, 0:QF], t[:, QF:HF])
                    state[i] = (xt, u, 2)

            # drain remaining stores, alternating so both rings share
            while pend["sync"] or pend["scalar"]:
                flush("sync", 1)
                flush("scalar", 1)

    nc.compile()
    _cache["nc"] = nc
    return nc


def run_spmd(x, mu, gamma, **spmd_kwargs):
    nc = build_nc()
    x = np.ascontiguousarray(x, dtype=np.float32)
    mu = np.ascontiguousarray(mu, dtype=np.float32)
    gamma = np.ascontiguousarray(gamma, dtype=np.float32)
    in_maps = [
        {"x": x[c * B_PER : (c + 1) * B_PER], "mu": mu, "gamma": gamma}
        for c in range(N_CORES)
    ]
    return run_bass_kernel_spmd(nc, in_maps, list(range(N_CORES)), **spmd_kwargs)


def kernel(x, mu, gamma):
    res = run_spmd(x, mu, gamma)
    out = np.concatenate([r["out"] for r in res.results], axis=0)
    return out
